# revision 1
# baseline (speedup 1.0000x reference)
"""Trainium2 Bass kernel for nn_Cnn_BiLSTM (embedding gather -> Conv1d+ReLU+MaxPool
-> BiLSTM(509 steps) -> attention pooling).

Sharding: data-parallel over the 128 paths across 8 NeuronCores (16 paths/core).
Each core: gathers its token embeddings, runs conv + pooling, runs the BiLSTM
recurrence for its 16 paths (both directions fused), and returns the final
hidden states.  The tiny attention-pooling epilogue runs on host over the
gathered 128x256 context matrix.

Key device-side design points:
  - embedding table uploaded as fp16; gather via indirect DMA, 8 rows per
    partition per call (1024 rows/call) to amortize SWDGE descriptor cost
  - token-major gather tiles transposed to [E, token] layout with PE transposes
  - conv as 6 accumulated matmuls per path (2 E-chunks x 3 taps)
  - LSTM state kept as [h_dim(partitions), paths(free)] per direction; the
    bias + x-projection (w_ih @ pooled) for 8-step windows are precomputed by
    matmuls directly into a PSUM window tile; each step's 4 recurrent matmuls
    (K=128, M=128, N=16) accumulate on top (start=False), so the gate
    pre-activations never touch a vector engine
  - sigmoid via tanh: sigma(x) = (tanh(x/2)+1)/2, scales folded into weights;
    cell update uses fused scalar_tensor_tensor ops; cell state kept fp32
  - fwd/bwd chains are independent and software-pipelined half a step apart
    so neither blocks the other in the in-order engine queues
"""

import numpy as np
import sys

if '/opt/trn_rl_repo' not in sys.path:
    sys.path.insert(0, '/opt/trn_rl_repo')

import concourse.bass as bass
import concourse.mybir as mybir
import concourse.tile as tile
from concourse import bacc
from concourse import bass_utils
from concourse.masks import make_identity

F16 = mybir.dt.float16
F32 = mybir.dt.float32
I32 = mybir.dt.int32
AF = mybir.ActivationFunctionType
OP = mybir.AluOpType

V, E, F, KS, H = 50000, 256, 128, 3, 128
NPATH, L = 128, 512
TCONV = L - KS + 1          # 510
T = TCONV - 1               # 509 steps after maxpool(2, stride 1)
NCORES = 8
PPC = NPATH // NCORES       # 16 paths per core
W = 8                       # gx window (steps)
LP = 512                    # per-path column stride in xT buffers
LPP = 520                   # per-path column stride in pooled (3 left-pad + 8 right-pad zeros)


def build_nc(t_steps=T, n_devices=NCORES):
    """Build the per-core Bass/Tile program. Same program on every core."""
    nwin = (t_steps + W - 1) // W
    nc = bacc.Bacc("TRN2", target_bir_lowering=False, debug=False,
                   num_devices=n_devices)

    pd_t = nc.dram_tensor("pd_t", [128, PPC * L // 128], I32, kind="ExternalInput")
    emb = nc.dram_tensor("emb", [V, E], F16, kind="ExternalInput")
    wp16 = nc.dram_tensor("wp16", [128, (6 + 8 + 8) * 128], F16, kind="ExternalInput")
    wp32 = nc.dram_tensor("wp32", [128, 1], F32, kind="ExternalInput")
    bsel = nc.dram_tensor("bsel", [4, 512 + 256], F16, kind="ExternalInput")
    ctx_o = nc.dram_tensor("ctx_o", [128, 32], F32, kind="ExternalOutput")

    NG = PPC * L // 128     # 64 gather tiles

    with tile.TileContext(nc) as tc:
        # ---- persistent SBUF ----
        with tc.tile_pool(name="persist", bufs=1) as pp:
            xT = pp.tile([128, 2 * PPC * LP], F16, tag="xT")
            pooled = pp.tile([128, PPC * LPP], F16, tag="pooled")
            wsb = pp.tile([128, 22 * 128], F16, tag="wsb")
            w32 = pp.tile([128, 1], F32, tag="w32")
            bs = pp.tile([4, 512 + 256], F16, tag="bs")
            ident = pp.tile([128, 128], F16, tag="ident")
            pd_sb = pp.tile([128, NG], I32, tag="pd")
            hT0 = pp.tile([128, 16], F16, tag="hT0")
            hT1 = pp.tile([128, 16], F16, tag="hT1")
            cC0 = pp.tile([128, 16], F32, tag="cC0")
            cC1 = pp.tile([128, 16], F32, tag="cC1")
            hO = pp.tile([128, 32], F32, tag="hO")

            cw = [wsb[:, (i) * 128:(i + 1) * 128] for i in range(6)]
            wh = [wsb[:, (6 + i) * 128:(7 + i) * 128] for i in range(8)]
            wi = [wsb[:, (14 + i) * 128:(15 + i) * 128] for i in range(8)]
            cb = w32[:, 0:1]
            selw = bs[:, 0:512]
            bmat = [bs[:, 512:640], bs[:, 640:768]]

            nc.sync.dma_start(pd_sb[:], pd_t.ap())
            nc.sync.dma_start(wsb[:], wp16.ap())
            nc.sync.dma_start(w32[:], wp32.ap())
            nc.sync.dma_start(bs[:], bsel.ap())
            make_identity(nc, ident[:])
            # zero pad columns: 3 on the left, 8 on the right of each path block
            pooled_pr = pooled[:].rearrange("e (p t) -> e p t", t=LPP)
            nc.gpsimd.memset(pooled_pr[:, :, 0:3], 0.0)
            nc.gpsimd.memset(pooled_pr[:, :, 3 + T:LPP], 0.0)
            nc.gpsimd.memset(hT0[:], 0.0)
            nc.gpsimd.memset(hT1[:], 0.0)
            nc.gpsimd.memset(cC0[:], 0.0)
            nc.gpsimd.memset(cC1[:], 0.0)

            # ---- phase 1: gather -> transpose -> conv -> pool ----
            with tc.tile_pool(name="gath", bufs=2) as pg, \
                 tc.tile_pool(name="tps", bufs=3, space="PSUM") as ptp, \
                 tc.tile_pool(name="cvp", bufs=4, space="PSUM") as pcv, \
                 tc.tile_pool(name="relu", bufs=3) as prl:
                GB = 8   # tokens gathered per partition per indirect DMA
                xg_cur = None
                for i in range(NG):
                    p, q = i // 4, i % 4
                    if i % GB == 0:
                        xg_cur = pg.tile([128, GB * E], F16, tag="xg")
                        nc.gpsimd.indirect_dma_start(
                            out=xg_cur[:], out_offset=None, in_=emb.ap(),
                            in_offset=bass.IndirectOffsetOnAxis(
                                ap=pd_sb[:, i:i + GB], axis=0),
                        )
                    xg = xg_cur[:, (i % GB) * E:(i % GB + 1) * E]
                    if True:  # transpose + relayout
                        tp = ptp.tile([128, 256], F16, tag="tp")
                        for c in (0, 1):
                            nc.tensor.transpose(tp[:, c * 128:(c + 1) * 128],
                                                xg[:, c * 128:(c + 1) * 128], ident[:])
                        dst = xT[:].rearrange("e (c n) -> e c n", c=2)[
                            :, :, p * LP + q * 128: p * LP + (q + 1) * 128]
                        srcr = tp[:].rearrange("e (c n) -> e c n", c=2)
                        if i % 2 == 0:
                            nc.vector.tensor_copy(dst, srcr)
                        else:
                            nc.scalar.copy(dst, srcr)
                    if q == 3:
                        cps = pcv.tile([128, TCONV], F32, tag="cps")
                        mm = 0
                        for c in (0, 1):
                            for k in range(KS):
                                nc.tensor.matmul(
                                    cps[:], lhsT=cw[c * 3 + k],
                                    rhs=xT[:, c * PPC * LP + p * LP + k:
                                           c * PPC * LP + p * LP + k + TCONV],
                                    start=(mm == 0), stop=(mm == 5))
                                mm += 1
                        rl = prl.tile([128, TCONV], F16, tag="rl")
                        nc.scalar.activation(rl[:], cps[:], AF.Relu, bias=cb, scale=1.0)
                        nc.vector.tensor_tensor(
                            out=pooled[:, p * LPP + 3: p * LPP + 3 + T],
                            in0=rl[:, 0:T], in1=rl[:, 1:TCONV], op=OP.max)

            # ---- phase 2: BiLSTM recurrence (two independent chains) ----
            pooled_r = pooled[:].rearrange("e (p t) -> e p t", t=LPP)

            with tc.tile_pool(name="gwin", bufs=2, space="PSUM") as pgw, \
                 tc.tile_pool(name="thsb", bufs=4) as pth, \
                 tc.tile_pool(name="small", bufs=4) as psm:

                hTs = [hT0[:], hT1[:]]
                hOs = [hO[:, 0:16], hO[:, 16:32]]
                cCs = [cC0[:], cC1[:]]
                nwin = (t_steps + W - 1) // W

                def emit_gwin(d, wn):
                    """Window tile [128, 512] = bias + x-projection for steps
                    [W*wn, W*wn+W), gate-major: col = g*128 + p*8 + j."""
                    t0 = W * wn
                    gw = pgw.tile([128, 512], F32, tag="gw%d" % d)
                    nc.tensor.matmul(gw[:], lhsT=bmat[d], rhs=selw,
                                     start=True, stop=False, skip_group_check=True)
                    for g in range(4):
                        if d == 0:
                            rhs = pooled_r[:, :, 3 + t0: 3 + t0 + W]
                        else:
                            rhs = pooled_r[:, :, 504 - t0: 504 - t0 + W]
                        nc.tensor.matmul(gw[:, g * 128:(g + 1) * 128],
                                         lhsT=wi[d * 4 + g], rhs=rhs,
                                         start=False, stop=False,
                                         skip_group_check=True)
                    return gw

                def mm_late(d, gw, t):
                    """Recurrent part accumulated into the window tile's
                    columns for step t (waits on this chain's h)."""
                    j = t % W if d == 0 else W - 1 - (t % W)
                    gwr = gw[:].rearrange("e (g p j) -> e g p j", g=4, j=W)
                    for g in range(4):
                        nc.tensor.matmul(gwr[:, g, :, j], lhsT=wh[d * 4 + g],
                                         rhs=hTs[d], start=False, stop=True,
                                         skip_group_check=True)

                def gate_tanh(d, gw, t):
                    j = t % W if d == 0 else W - 1 - (t % W)
                    gwr = gw[:].rearrange("e (g p j) -> e g p j", g=4, j=W)
                    th = pth.tile([128, 64], F16, tag="th%d" % d)
                    nc.scalar.activation(
                        th[:].rearrange("e (g p) -> e g p", g=4),
                        gwr[:, :, :, j], AF.Tanh)
                    return th

                def cell_h(d, t, th):
                    th_r = th[:].rearrange("e (g p) -> e g p", g=4)
                    t1 = psm.tile([128, 16], F32, tag="t1%d" % d)
                    t2 = psm.tile([128, 16], F32, tag="t2%d" % d)
                    # t1 = (th_F + 1) * c~ ; t2 = (th_I + 1) * th_G
                    nc.vector.scalar_tensor_tensor(
                        out=t1[:], in0=th_r[:, 1], scalar=1.0, in1=cCs[d],
                        op0=OP.add, op1=OP.mult)
                    nc.vector.scalar_tensor_tensor(
                        out=t2[:], in0=th_r[:, 0], scalar=1.0, in1=th_r[:, 2],
                        op0=OP.add, op1=OP.mult)
                    # c~ = 0.5*t1 + t2
                    nc.vector.scalar_tensor_tensor(
                        out=cCs[d], in0=t1[:], scalar=0.5, in1=t2[:],
                        op0=OP.mult, op1=OP.add)
                    thc = psm.tile([128, 16], F16, tag="thc%d" % d)
                    nc.scalar.activation(thc[:], cCs[d], AF.Tanh, scale=0.5)
                    # h~ = (th_O + 1) * tanh(c)
                    dst = hTs[d] if t + 1 < t_steps else hOs[d]
                    nc.vector.scalar_tensor_tensor(
                        out=dst, in0=th_r[:, 3], scalar=1.0, in1=thc[:],
                        op0=OP.add, op1=OP.mult)

                # Software-pipelined: bwd chain runs a half step behind fwd.
                gw_cur = [emit_gwin(0, 0), emit_gwin(1, 0)]
                gw_nxt = [None, None]
                th_b_prev = None
                for t in range(t_steps):
                    wn, j = t // W, t % W
                    if j == 0 and wn > 0:
                        gw_cur = gw_nxt
                    if j == 0 and wn + 1 < nwin:
                        gw_nxt = [emit_gwin(0, wn + 1), emit_gwin(1, wn + 1)]
                    if th_b_prev is not None:
                        cell_h(1, t - 1, th_b_prev)
                    mm_late(0, gw_cur[0], t)
                    th_f = gate_tanh(0, gw_cur[0], t)
                    mm_late(1, gw_cur[1], t)
                    th_b_prev = gate_tanh(1, gw_cur[1], t)
                    cell_h(0, t, th_f)
                if th_b_prev is not None:
                    cell_h(1, t_steps - 1, th_b_prev)

                nc.sync.dma_start(ctx_o.ap(), hO[:])

    nc.compile()
    return nc


def _prep_inputs(path_data, emb_A, conv_w, conv_b,
                 w_ih_f, w_hh_f, b_ih_f, b_hh_f,
                 w_ih_b, w_hh_b, b_ih_b, b_hh_b):
    """Host-side packing; returns per-core input maps."""
    emb16 = np.ascontiguousarray(emb_A.astype(np.float16))

    # conv lhsT tiles: cw[c*3+k][e, f] = conv_w[f, 128c+e, k]
    cw = np.zeros((6, 128, 128), np.float16)
    for c in range(2):
        for k in range(KS):
            cw[c * 3 + k] = conv_w[:, c * 128:(c + 1) * 128, k].T

    sg = np.array([0.5, 0.5, 1.0, 0.5], np.float32)  # i, f, g, o
    wh = np.zeros((8, 128, 128), np.float16)
    wi = np.zeros((8, 128, 128), np.float16)
    bwv = np.zeros((8, 128), np.float32)
    for d, (wihd, whhd, bihd, bhhd) in enumerate(
            ((w_ih_f, w_hh_f, b_ih_f, b_hh_f), (w_ih_b, w_hh_b, b_ih_b, b_hh_b))):
        for g in range(4):
            grp = d * 4 + g
            rows = slice(g * H, (g + 1) * H)
            wh[grp] = (0.5 * sg[g] * whhd[rows, :]).T  # [hin, hout]
            wi[grp] = (sg[g] * wihd[rows, :]).T        # [f, hout]
            bwv[grp] = sg[g] * (bihd[rows] + bhhd[rows])

    wp16 = np.concatenate([cw.transpose(1, 0, 2).reshape(128, 6 * 128),
                           wh.transpose(1, 0, 2).reshape(128, 8 * 128),
                           wi.transpose(1, 0, 2).reshape(128, 8 * 128)], axis=1)
    wp16 = np.ascontiguousarray(wp16)
    wp32 = np.ascontiguousarray(conv_b.reshape(128, 1).astype(np.float32))

    # bsel: [4, 512] gate-block selector | bias matrices for fwd/bwd as [4, 128]
    selw = np.zeros((4, 512), np.float16)
    for g in range(4):
        selw[g, g * 128:(g + 1) * 128] = 1.0
    bsel = np.concatenate(
        [selw, bwv[0:4].astype(np.float16), bwv[4:8].astype(np.float16)], axis=1)
    bsel = np.ascontiguousarray(bsel)

    in_maps = []
    for c in range(NCORES):
        pdl = path_data[c * PPC:(c + 1) * PPC]          # [16, 512]
        # pd_t[jrow, i] = pdl[i//4, (i%4)*128 + jrow]
        pd = pdl.reshape(PPC, 4, 128).transpose(2, 0, 1).reshape(128, PPC * 4)
        in_maps.append({
            "pd_t": np.ascontiguousarray(pd.astype(np.int32)),
            "emb": emb16,
            "wp16": wp16,
            "wp32": wp32,
            "bsel": bsel,
        })
    return in_maps


_CACHE = {}


def _get_runner(t_steps=T):
    if t_steps not in _CACHE:
        _CACHE[t_steps] = build_nc(t_steps)
    return _CACHE[t_steps]


def run_device(in_maps, t_steps=T):
    nc = _get_runner(t_steps)
    res = bass_utils.run_bass_kernel_spmd(nc, in_maps, core_ids=list(range(NCORES)))
    return res.results


def host_attention(context, u0, d1_w, d1_b, d2_w, d2_b):
    context = context.astype(np.float32)
    u = u0.astype(np.float32)
    P = context.shape[0]
    for _ in range(2):
        cat = np.concatenate([context, np.broadcast_to(u, (P, E))], axis=1)
        tt = np.tanh(cat @ d1_w.T + d1_b)
        score = (tt @ d2_w.T + d2_b).reshape(-1)
        score = score - score.max()
        alpha = np.exp(score)
        alpha /= alpha.sum()
        o = (alpha[:, None] * context).sum(axis=0)
        u = np.concatenate([u, o]) @ d1_w.T + d1_b
    u = np.maximum(u, 0.0)
    pred = 1.0 / (1.0 + np.exp(-(u @ d2_w.T + d2_b)))
    return np.float32(pred.squeeze())


def kernel(path_data, query, emb_A, emb_B, conv_w, conv_b,
           w_ih_f, w_hh_f, b_ih_f, b_hh_f,
           w_ih_b, w_hh_b, b_ih_b, b_hh_b,
           d1_w, d1_b, d2_w, d2_b):
    path_data = np.asarray(path_data)
    in_maps = _prep_inputs(np.asarray(path_data), np.asarray(emb_A),
                           np.asarray(conv_w), np.asarray(conv_b),
                           np.asarray(w_ih_f), np.asarray(w_hh_f),
                           np.asarray(b_ih_f), np.asarray(b_hh_f),
                           np.asarray(w_ih_b), np.asarray(w_hh_b),
                           np.asarray(b_ih_b), np.asarray(b_hh_b))
    results = run_device(in_maps)
    context = np.zeros((NPATH, E), np.float32)
    for c in range(NCORES):
        ho = results[c]["ctx_o"]          # [128, 32] fp32, = 2*h
        context[c * PPC:(c + 1) * PPC, 0:H] = 0.5 * ho[:, 0:PPC].T
        context[c * PPC:(c + 1) * PPC, H:E] = 0.5 * ho[:, PPC:2 * PPC].T
    u0 = np.asarray(emb_B)[int(np.asarray(query))]
    return host_attention(context, u0, np.asarray(d1_w), np.asarray(d1_b),
                          np.asarray(d2_w), np.asarray(d2_b))


def bench(in_maps, iters=10, t_steps=T):
    """Time repeated executions with device-resident inputs. Returns list of
    per-call wall times (s). First call includes compile."""
    import time
    import jax
    import numpy as _np
    from jax.sharding import Mesh, PartitionSpec
    from jax.experimental.shard_map import shard_map
    from concourse import bass2jax as B
    from concourse import mybir as _mb

    nc = _get_runner(t_steps)
    B.install_neuronx_cc_hook()
    partition_name = nc.partition_id_tensor.name if nc.partition_id_tensor else None
    in_names, out_names, out_avals, zero_outs = [], [], [], []
    for alloc in nc.m.functions[0].allocations:
        if not isinstance(alloc, B.mybir.MemoryLocationSet):
            continue
        name = alloc.memorylocations[0].name
        if alloc.kind == "ExternalInput":
            if name != partition_name:
                in_names.append(name)
        elif alloc.kind == "ExternalOutput":
            out_names.append(name)
            shape = tuple(alloc.tensor_shape)
            dtype = B.mybir.dt.np(alloc.dtype)
            out_avals.append(jax.core.ShapedArray(shape, dtype))
            zero_outs.append(_np.zeros(shape, dtype))
    n_params = len(in_names)
    n_outs = len(out_avals)
    all_names = in_names + out_names + ([partition_name] if partition_name else [])
    donate = tuple(range(n_params, n_params + n_outs))

    def _body(*args):
        operands = list(args)
        if partition_name is not None:
            operands.append(B.partition_id_tensor())
        return tuple(B._bass_exec_p.bind(
            *operands, out_avals=tuple(out_avals), in_names=tuple(all_names),
            out_names=tuple(out_names), lowering_input_output_aliases=(),
            sim_require_finite=True, sim_require_nnan=True, nc=nc))

    devices = jax.devices()[:NCORES]
    mesh = Mesh(_np.asarray(devices), ("core",))
    sharded = jax.jit(
        shard_map(_body, mesh=mesh,
                  in_specs=(PartitionSpec("core"),) * (n_params + n_outs),
                  out_specs=(PartitionSpec("core"),) * n_outs,
                  check_rep=False),
        donate_argnums=donate, keep_unused=True)
    concat_in = [jax.device_put(
        _np.concatenate([_np.asarray(in_maps[c][nm]) for c in range(NCORES)], axis=0))
        for nm in in_names]
    for a in concat_in:
        a.block_until_ready()
    times = []
    for _ in range(iters):
        zs = [_np.zeros((NCORES * z.shape[0], *z.shape[1:]), z.dtype) for z in zero_outs]
        t0 = time.time()
        outs = sharded(*concat_in, *zs)
        jax.block_until_ready(outs)
        times.append(time.time() - t0)
    return times

